# revision 43
# baseline (speedup 1.0000x reference)
"""KNN loss kernel for Trainium2 (Bass/Tile), data-parallel over batch.

Strategy (one batch per NeuronCore):
  1. HOST: sort each batch's points by x-coordinate. All neighbors within
     RADIUS=0.25 of a point then lie in a narrow contiguous rank band
     (~<=470 ranks for N(0,1) data), so each 128-row block only needs a
     ~250-1100 wide column band of the NxN distance matrix (~5x fewer
     elements than the full 4096).
  2. Coordinates are quantized to a 2^-8 grid and encoded in fp16 so the
     PE matmul (1 cycle/row vs 4 for fp32) produces
        w = R^2 - d^2  EXACTLY as a multiple of 2^-16 in f32 PSUM.
     A second 2-row matmul accumulates j*2^-28 (j = sorted column id) into
     the same PSUM bank: for any in-radius pair |w| < 2^-4 the sum
     w + j*2^-28 is exact in f32, so the neighbor INDEX rides for free in
     the low mantissa bits (no DVE pack pass and no max_index scans).
  3. Act engine copies PSUM->SBUF; DVE takes top-8 of 3 mod-3 strided
     slices (de-clustered: sorted neighbors are rank-contiguous but spread
     uniformly mod 3) and ships all 24 packed f32 candidates per row (no
     device-side merge: index extraction must happen on host anyway since
     the idx bit position floats with the f32 exponent, so a device-side
     AND cannot recover it). Outputs are DMA'd in groups of GRP blocks;
     the first 6 blocks' operands ride in one packed "head" DMA so the
     pipeline fills early.
  4. HOST: top-16 of 24 per row, decode indices, force slot 0 to self,
     patch grid-coincident pairs (w ties at R^2 where the idx bits no
     longer fit in f32), map through the sort permutation, gather flows,
     L1 + mean.

Cost-model breakdown at 40.9us total: DVE max8 33.2us busy (the floor:
InstMax has no 2x/4x DVE modes, 1.04ns/elem * sum(widths)=25.9k elems +
96 * ~105ns overheads), Act 29.1us, PE 24us, ~4us head + ~3us tail (DMA
generation + semaphore-propagation chains).
"""

from contextlib import ExitStack

import numpy as np

import concourse.bacc as bacc
import concourse.mybir as mybir
import concourse.tile as tile
from concourse.bass_utils import run_bass_kernel_spmd

B = 8
N = 4096
K = 16
RADIUS = 0.25
R2 = RADIUS * RADIUS
BLK = 128
NBLK = N // BLK  # 32
NSLICE = 3
NCAND = 8 * NSLICE
MARGIN = 2
GRP = 4  # blocks per batched output DMA
GBITS = 8  # coordinate grid 2^-8
F16 = mybir.dt.float16
F32 = mybir.dt.float32

# Window table for the canonical seed-0 input (used when _get_program() is
# called without runtime data, e.g. by the timeline simulator). kernel()
# recomputes windows from its actual input and compiles a fresh program if
# they differ.
DEFAULT_OFFS = (0, 0, 20, 136, 222, 318, 418, 514, 644, 756, 878, 990, 1095,
                1216, 1330, 1473, 1603, 1735, 1860, 1961, 2124, 2263, 2414,
                2565, 2703, 2849, 3006, 3173, 3323, 3486, 3664, 3862)
DEFAULT_WIDTHS = (228, 420, 600, 624, 708, 774, 828, 894, 888, 924, 936, 966,
                  1014, 1026, 1056, 1026, 1020, 1014, 1020, 1074, 1002, 978,
                  936, 888, 870, 834, 774, 696, 654, 582, 432, 234)


def _windows_from_sorted(xs_all):
    """Per-block [offset, width] bands covering every in-radius pair, from
    the sorted x-coordinates of all batches. Width is a multiple of NSLICE."""
    spans = np.zeros((len(xs_all), NBLK), dtype=np.int64)
    for b, xi in enumerate(xs_all):
        lo = np.searchsorted(xi, xi - (RADIUS + 1e-7))
        hi = np.searchsorted(xi, xi + (RADIUS + 1e-7))
        for I in range(NBLK):
            r0, r1 = I * BLK, (I + 1) * BLK
            spans[b, I] = max(r0 - lo[r0:r1].min(), hi[r0:r1].max() - r1)
    offs, widths = [], []
    mult = 2 * NSLICE  # fold halves must each be a multiple of NSLICE
    for I in range(NBLK):
        h = int(spans[:, I].max()) + MARGIN
        o = max(0, I * BLK - h)
        e = min(N, (I + 1) * BLK + h)
        c = ((e - o + mult - 1) // mult) * mult
        e = min(N, o + c)
        o = e - c
        offs.append(o)
        widths.append(c)
    return tuple(offs), tuple(widths)


def _build_program(offs, widths):
    nc = bacc.Bacc(
        "TRN2",
        target_bir_lowering=False,
        debug=False,
        num_devices=B,
    )
    # lhsT/rhs: the 7 w-term rows; lhsT2/rhs2: the 2 index-packing rows
    lhsT_d = nc.dram_tensor("lhsT", [7, N], F16, kind="ExternalInput").ap()
    rhs_d = nc.dram_tensor("rhs", [7, N], F16, kind="ExternalInput").ap()
    lhsT2_d = nc.dram_tensor("lhsT2", [2, N], F16, kind="ExternalInput").ap()
    rhs2_d = nc.dram_tensor("rhs2", [2, N], F16, kind="ExternalInput").ap()
    # head: block 0's four operand groups packed into one tensor (one DMA
    # generation before the first matmul instead of four)
    head_d = nc.dram_tensor("head", [7, 4096], F16, kind="ExternalInput").ap()
    t16_d = nc.dram_tensor("t16", [N, NCAND], F32, kind="ExternalOutput").ap()

    cpad = max(widths)
    cpad = ((cpad + 511) // 512) * 512  # PSUM bank multiple

    with tile.TileContext(nc) as tc:
        with ExitStack() as ctx:
            const = ctx.enter_context(tc.tile_pool(name="const", bufs=1))
            psum = ctx.enter_context(tc.tile_pool(name="psum", bufs=2, space="PSUM"))
            wpool = ctx.enter_context(tc.tile_pool(name="w", bufs=3))
            small = ctx.enter_context(tc.tile_pool(name="small", bufs=8))

            lhsT = const.tile([7, N], F16)
            rhs = const.tile([7, N], F16)
            lhsT2 = const.tile([2, N], F16)
            rhs2 = const.tile([2, N], F16)
            head = const.tile([7, 4096], F16)
            nc.sync.dma_start(head[:], head_d[:])
            nc.sync.dma_start(rhs2[:], rhs2_d[:])
            nc.sync.dma_start(lhsT2[:], lhsT2_d[:])
            nc.sync.dma_start(rhs[:], rhs_d[:])
            nc.sync.dma_start(lhsT[:], lhsT_d[:])

            for I in range(NBLK):
                o, c = offs[I], widths[I]
                ps = psum.tile([BLK, cpad], F32)
                if (I + 1) * BLK <= 768 and o + c <= 1280:
                    # early blocks read from the packed head tile (one DMA)
                    lw = head[0:7, I * BLK : (I + 1) * BLK]
                    li = head[0:2, 2048 + I * BLK : 2048 + (I + 1) * BLK]
                    rh = head[0:7, 768 + o : 768 + o + c]
                    r2 = head[0:2, 2816 + o : 2816 + o + c]
                else:
                    lw = lhsT[:, I * BLK : (I + 1) * BLK]
                    li = lhsT2[:, I * BLK : (I + 1) * BLK]
                    rh = rhs[:, o : o + c]
                    r2 = rhs2[:, o : o + c]
                p0 = 0
                while p0 < c:
                    pw = min(512, c - p0)
                    # w-terms, then index terms accumulated into the same bank
                    nc.tensor.matmul(
                        ps[:, p0 : p0 + pw],
                        lw,
                        rh[:, p0 : p0 + pw],
                        start=True,
                        stop=False,
                    )
                    nc.tensor.matmul(
                        ps[:, p0 : p0 + pw],
                        li,
                        r2[:, p0 : p0 + pw],
                        start=False,
                        stop=True,
                    )
                    p0 += pw
                wsb = wpool.tile([BLK, cpad], F32)
                nc.scalar.copy(wsb[:, 0:c], ps[:, 0:c])
                g = I % GRP
                if g == 0:
                    cand = small.tile([BLK, GRP * NCAND], F32, tag="cand")
                    cand_hold = cand
                else:
                    cand = cand_hold
                for s in range(NSLICE):
                    nc.vector.max(
                        cand[:, g * NCAND + s * 8 : g * NCAND + (s + 1) * 8],
                        wsb[:, s : c : NSLICE],
                    )
                if g == GRP - 1:
                    # one batched DMA for GRP blocks: DRAM rows (I-g)*128 ..
                    dst = t16_d[(I - g) * BLK : (I + 1) * BLK, :].rearrange(
                        "(grp p) k -> p grp k", grp=GRP
                    )
                    src = cand[:].rearrange("p (grp k) -> p grp k", grp=GRP)
                    nc.sync.dma_start(dst, src)
    nc.compile()
    return nc


_NC_CACHE = {}


def _get_program(offs=DEFAULT_OFFS, widths=DEFAULT_WIDTHS):
    key = (tuple(offs), tuple(widths))
    if key not in _NC_CACHE:
        _NC_CACHE[key] = _build_program(*key)
    return _NC_CACHE[key]


def _encode(xq, sq_units):
    """fp16 feature rows for one batch of sorted quantized coords.
    xq: [N,3] integer grid coords; sq_units: [N] = sum(xq^2) (units 2^-16)."""
    G = 2.0**-GBITS
    m = np.round(R2 * 2**16).astype(np.int64) - sq_units  # (R2-sq)*2^16
    a = np.round(m / 4096.0)
    bb = m - a * 4096
    am = -sq_units  # -sq * 2^16
    al = np.round(am / 4096.0)
    be = am - al * 4096
    assert np.abs(a).max() <= 2047 and np.abs(al).max() <= 2047
    assert np.abs(bb).max() <= 2048 and np.abs(be).max() <= 2048
    j = np.arange(N, dtype=np.int64)
    ones = np.ones(N)
    lhsT = np.stack([
        xq[:, 0] * G, xq[:, 1] * G, xq[:, 2] * G,
        a * 2.0**-4, bb * 2.0**-16,
        ones, ones,
    ]).astype(np.float16)
    rhs = np.stack([
        2 * xq[:, 0] * G, 2 * xq[:, 1] * G, 2 * xq[:, 2] * G,
        ones, ones,
        al * 2.0**-4, be * 2.0**-16,
    ]).astype(np.float16)
    # idx*2^-28 split across both operands: rhs values stay in fp16 normal
    # range (plain j*2^-22/2^-28 would be subnormal and lose low bits)
    lhsT2 = np.stack([ones * 2.0**-8, ones * 2.0**-14]).astype(np.float16)
    rhs2 = np.stack([
        (j >> 6) * 2.0**-14, (j & 63) * 2.0**-14,
    ]).astype(np.float16)
    head = np.zeros((7, 4096), dtype=np.float16)
    head[:, 0:768] = lhsT[:, 0:768]
    head[:, 768:2048] = rhs[:, 0:1280]
    head[0:2, 2048:2816] = lhsT2[:, 0:768]
    head[0:2, 2816:4096] = rhs2[:, 0:1280]
    return (np.ascontiguousarray(lhsT), np.ascontiguousarray(rhs),
            np.ascontiguousarray(lhsT2), np.ascontiguousarray(rhs2),
            head)


def _prep(pc):
    """Sort, quantize, window, and encode all batches."""
    pc = np.asarray(pc, dtype=np.float32)
    perms, xqs, sqs, xs_list = [], [], [], []
    for b in range(B):
        perm = np.argsort(pc[b][:, 0], kind="stable")
        xs = pc[b][perm].astype(np.float64)
        xq = np.round(xs * (2**GBITS))
        assert np.abs(xq).max() <= 2047
        perms.append(perm)
        xqs.append(xq)
        sqs.append((xq * xq).sum(-1).astype(np.int64))
        xs_list.append(xs[:, 0])
    offs, widths = _windows_from_sorted(xs_list)
    return perms, xqs, sqs, offs, widths


def run_device(pc, trace: bool = False):
    """Returns (list of per-core t16 [N,K] f32 packed winners, results,
    per-batch perms, per-batch xq)."""
    perms, xqs, sqs, offs, widths = _prep(pc)
    in_maps = []
    for b in range(B):
        lhsT, rhs, lhsT2, rhs2, head = _encode(xqs[b], sqs[b])
        in_maps.append({"lhsT": lhsT, "rhs": rhs, "lhsT2": lhsT2,
                        "rhs2": rhs2, "head": head})
    nc = _get_program(offs, widths)
    res = run_bass_kernel_spmd(nc, in_maps, core_ids=list(range(B)), trace=trace)
    t16s = [res.results[b]["t16"] for b in range(B)]
    return t16s, res, perms, xqs


def kernel(pc: np.ndarray, flow: np.ndarray) -> np.ndarray:
    pc = np.asarray(pc, dtype=np.float32)
    flow = np.asarray(flow, dtype=np.float32)
    t16s, _, perms, xqs = run_device(pc)
    total = 0.0
    rid = np.arange(N, dtype=np.int64)
    for b in range(B):
        cand = t16s[b].astype(np.float64)  # [N, NCAND]
        w64 = -np.partition(-cand, K - 1, axis=1)[:, :K]  # top-16 of 24
        w64 = np.sort(w64, axis=1)[:, ::-1]
        wg = np.floor(w64 * 2.0**16) * 2.0**-16
        jrec = np.round((w64 - wg) * 2.0**28).astype(np.int64)
        sel = w64 > 0
        res = np.where(sel, np.clip(jrec, 0, N - 1), rid[:, None])
        res[:, 0] = rid
        # grid-coincident pairs tie at w=R^2 where idx bits no longer fit in
        # f32; restore both partners exactly.
        xq = xqs[b].astype(np.int64)
        key = ((xq[:, 0] + 4096) << 26) + ((xq[:, 1] + 4096) << 13) + (xq[:, 2] + 4096)
        order = np.argsort(key, kind="stable")
        ks = key[order]
        for t in np.nonzero(ks[1:] == ks[:-1])[0]:
            i, j = order[t], order[t + 1]
            res[i, 1] = j
            res[j, 1] = i
        fs = flow[b][perms[b]].astype(np.float64)
        nn = fs[res]
        total += np.abs(fs[:, None, :] - nn).sum()
    return np.float32(total / (B * N * K))


# revision 49
# speedup vs baseline: 1.0077x; 1.0077x over previous
"""KNN loss kernel for Trainium2 (Bass/Tile), data-parallel over batch.

Strategy (one batch per NeuronCore):
  1. HOST: sort each batch's points by x-coordinate. All neighbors within
     RADIUS=0.25 of a point then lie in a narrow contiguous rank band
     (~<=470 ranks for N(0,1) data), so each 128-row block only needs a
     ~250-1100 wide column band of the NxN distance matrix (~5x fewer
     elements than the full 4096).
  2. Coordinates are quantized to a 2^-8 grid and encoded in fp16 so the
     PE matmul (1 cycle/row vs 4 for fp32) produces
        w = R^2 - d^2  EXACTLY as a multiple of 2^-16 in f32 PSUM.
     A second 2-row matmul accumulates j*2^-28 (j = sorted column id) into
     the same PSUM bank: for any in-radius pair |w| < 2^-4 the sum
     w + j*2^-28 is exact in f32, so the neighbor INDEX rides for free in
     the low mantissa bits (no DVE pack pass and no max_index scans).
  3. Act engine copies PSUM->SBUF; DVE takes top-8 of 3 mod-3 strided
     slices (de-clustered: sorted neighbors are rank-contiguous but spread
     uniformly mod 3) and ships all 24 packed f32 candidates per row (no
     device-side merge: index extraction must happen on host anyway since
     the idx bit position floats with the f32 exponent, so a device-side
     AND cannot recover it). Outputs are DMA'd in groups of GRP blocks;
     the first 6 blocks' operands ride in one packed "head" DMA so the
     pipeline fills early.
  4. HOST: top-16 of 24 per row, decode indices, force slot 0 to self,
     patch grid-coincident pairs (w ties at R^2 where the idx bits no
     longer fit in f32), map through the sort permutation, gather flows,
     L1 + mean.

Cost-model breakdown at 40.9us total: DVE max8 33.2us busy (the floor:
InstMax has no 2x/4x DVE modes, 1.04ns/elem * sum(widths)=25.9k elems +
96 * ~105ns overheads), Act 29.1us, PE 24us, ~4us head + ~3us tail (DMA
generation + semaphore-propagation chains).
"""

from contextlib import ExitStack

import numpy as np

import concourse.bacc as bacc
import concourse.mybir as mybir
import concourse.tile as tile
from concourse.bass_utils import run_bass_kernel_spmd

B = 8
N = 4096
K = 16
RADIUS = 0.25
R2 = RADIUS * RADIUS
BLK = 128
NBLK = N // BLK  # 32
NSLICE = 3
NCAND = 8 * NSLICE
MARGIN = 2
GRP = 4  # blocks per batched output DMA
GBITS = 8  # coordinate grid 2^-8
F16 = mybir.dt.float16
F32 = mybir.dt.float32

# Window table for the canonical seed-0 input (used when _get_program() is
# called without runtime data, e.g. by the timeline simulator). kernel()
# recomputes windows from its actual input and compiles a fresh program if
# they differ.
DEFAULT_OFFS = (0, 0, 20, 136, 222, 318, 418, 514, 644, 756, 878, 990, 1095,
                1216, 1330, 1473, 1603, 1735, 1860, 1961, 2124, 2263, 2414,
                2565, 2703, 2849, 3006, 3173, 3323, 3486, 3664, 3862)
DEFAULT_WIDTHS = (228, 417, 600, 624, 708, 774, 828, 894, 888, 921, 933, 966,
                  1011, 1026, 1053, 1023, 1020, 1011, 1017, 1071, 1002, 978,
                  933, 888, 867, 831, 774, 696, 651, 582, 432, 234)


def _windows_from_sorted(xs_all):
    """Per-block [offset, width] bands covering every in-radius pair, from
    the sorted x-coordinates of all batches. Width is a multiple of NSLICE."""
    spans = np.zeros((len(xs_all), NBLK), dtype=np.int64)
    for b, xi in enumerate(xs_all):
        lo = np.searchsorted(xi, xi - (RADIUS + 1e-7))
        hi = np.searchsorted(xi, xi + (RADIUS + 1e-7))
        for I in range(NBLK):
            r0, r1 = I * BLK, (I + 1) * BLK
            spans[b, I] = max(r0 - lo[r0:r1].min(), hi[r0:r1].max() - r1)
    offs, widths = [], []
    mult = NSLICE
    for I in range(NBLK):
        h = int(spans[:, I].max()) + MARGIN
        o = max(0, I * BLK - h)
        e = min(N, (I + 1) * BLK + h)
        c = ((e - o + mult - 1) // mult) * mult
        e = min(N, o + c)
        o = e - c
        offs.append(o)
        widths.append(c)
    return tuple(offs), tuple(widths)


def _build_program(offs, widths):
    nc = bacc.Bacc(
        "TRN2",
        target_bir_lowering=False,
        debug=False,
        num_devices=B,
    )
    # lhsT/rhs: the 7 w-term rows; lhsT2/rhs2: the 2 index-packing rows
    lhsT_d = nc.dram_tensor("lhsT", [7, N], F16, kind="ExternalInput").ap()
    rhs_d = nc.dram_tensor("rhs", [7, N], F16, kind="ExternalInput").ap()
    lhsT2_d = nc.dram_tensor("lhsT2", [2, N], F16, kind="ExternalInput").ap()
    rhs2_d = nc.dram_tensor("rhs2", [2, N], F16, kind="ExternalInput").ap()
    # head: block 0's four operand groups packed into one tensor (one DMA
    # generation before the first matmul instead of four)
    head_d = nc.dram_tensor("head", [7, 5120], F16, kind="ExternalInput").ap()
    t16_d = nc.dram_tensor("t16", [N, NCAND], F32, kind="ExternalOutput").ap()

    cpad = max(widths)
    cpad = ((cpad + 511) // 512) * 512  # PSUM bank multiple

    with tile.TileContext(nc) as tc:
        with ExitStack() as ctx:
            const = ctx.enter_context(tc.tile_pool(name="const", bufs=1))
            psum = ctx.enter_context(tc.tile_pool(name="psum", bufs=2, space="PSUM"))
            wpool = ctx.enter_context(tc.tile_pool(name="w", bufs=3))
            small = ctx.enter_context(tc.tile_pool(name="small", bufs=8))

            lhsT = const.tile([7, N], F16)
            rhs = const.tile([7, N], F16)
            lhsT2 = const.tile([2, N], F16)
            rhs2 = const.tile([2, N], F16)
            head = const.tile([7, 5120], F16)
            nc.sync.dma_start(head[:], head_d[:])
            nc.sync.dma_start(rhs2[:], rhs2_d[:])
            nc.sync.dma_start(lhsT2[:], lhsT2_d[:])
            nc.sync.dma_start(rhs[:], rhs_d[:])
            nc.sync.dma_start(lhsT[:], lhsT_d[:])

            for I in range(NBLK):
                o, c = offs[I], widths[I]
                if c <= 512:
                    ps = psum.tile([BLK, 512], F32, tag="narrow")
                else:
                    ps = psum.tile([BLK, cpad], F32, tag="wide")
                if (I + 1) * BLK <= 1024 and o + c <= 1536:
                    # early blocks read from the packed head tile (one DMA)
                    lw = head[0:7, I * BLK : (I + 1) * BLK]
                    li = head[0:2, 2560 + I * BLK : 2560 + (I + 1) * BLK]
                    rh = head[0:7, 1024 + o : 1024 + o + c]
                    r2 = head[0:2, 3584 + o : 3584 + o + c]
                else:
                    lw = lhsT[:, I * BLK : (I + 1) * BLK]
                    li = lhsT2[:, I * BLK : (I + 1) * BLK]
                    rh = rhs[:, o : o + c]
                    r2 = rhs2[:, o : o + c]
                p0 = 0
                while p0 < c:
                    pw = min(512, c - p0)
                    # w-terms, then index terms accumulated into the same bank
                    nc.tensor.matmul(
                        ps[:, p0 : p0 + pw],
                        lw,
                        rh[:, p0 : p0 + pw],
                        start=True,
                        stop=False,
                    )
                    nc.tensor.matmul(
                        ps[:, p0 : p0 + pw],
                        li,
                        r2[:, p0 : p0 + pw],
                        start=False,
                        stop=True,
                    )
                    p0 += pw
                wsb = wpool.tile([BLK, cpad], F32)
                nc.scalar.copy(wsb[:, 0:c], ps[:, 0:c])
                g = I % GRP
                if g == 0:
                    cand = small.tile([BLK, GRP * NCAND], F32, tag="cand")
                    cand_hold = cand
                else:
                    cand = cand_hold
                for s in range(NSLICE):
                    nc.vector.max(
                        cand[:, g * NCAND + s * 8 : g * NCAND + (s + 1) * 8],
                        wsb[:, s : c : NSLICE],
                    )
                if g == GRP - 1:
                    # one batched DMA for GRP blocks: DRAM rows (I-g)*128 ..
                    dst = t16_d[(I - g) * BLK : (I + 1) * BLK, :].rearrange(
                        "(grp p) k -> p grp k", grp=GRP
                    )
                    src = cand[:].rearrange("p (grp k) -> p grp k", grp=GRP)
                    nc.sync.dma_start(dst, src)
    nc.compile()
    return nc


_NC_CACHE = {}


def _get_program(offs=DEFAULT_OFFS, widths=DEFAULT_WIDTHS):
    key = (tuple(offs), tuple(widths))
    if key not in _NC_CACHE:
        _NC_CACHE[key] = _build_program(*key)
    return _NC_CACHE[key]


def _encode(xq, sq_units):
    """fp16 feature rows for one batch of sorted quantized coords.
    xq: [N,3] integer grid coords; sq_units: [N] = sum(xq^2) (units 2^-16)."""
    G = 2.0**-GBITS
    m = np.round(R2 * 2**16).astype(np.int64) - sq_units  # (R2-sq)*2^16
    a = np.round(m / 4096.0)
    bb = m - a * 4096
    am = -sq_units  # -sq * 2^16
    al = np.round(am / 4096.0)
    be = am - al * 4096
    assert np.abs(a).max() <= 2047 and np.abs(al).max() <= 2047
    assert np.abs(bb).max() <= 2048 and np.abs(be).max() <= 2048
    j = np.arange(N, dtype=np.int64)
    ones = np.ones(N)
    lhsT = np.stack([
        xq[:, 0] * G, xq[:, 1] * G, xq[:, 2] * G,
        a * 2.0**-4, bb * 2.0**-16,
        ones, ones,
    ]).astype(np.float16)
    rhs = np.stack([
        2 * xq[:, 0] * G, 2 * xq[:, 1] * G, 2 * xq[:, 2] * G,
        ones, ones,
        al * 2.0**-4, be * 2.0**-16,
    ]).astype(np.float16)
    # idx*2^-28 split across both operands: rhs values stay in fp16 normal
    # range (plain j*2^-22/2^-28 would be subnormal and lose low bits)
    lhsT2 = np.stack([ones * 2.0**-8, ones * 2.0**-14]).astype(np.float16)
    rhs2 = np.stack([
        (j >> 6) * 2.0**-14, (j & 63) * 2.0**-14,
    ]).astype(np.float16)
    head = np.zeros((7, 5120), dtype=np.float16)
    head[:, 0:1024] = lhsT[:, 0:1024]
    head[:, 1024:2560] = rhs[:, 0:1536]
    head[0:2, 2560:3584] = lhsT2[:, 0:1024]
    head[0:2, 3584:5120] = rhs2[:, 0:1536]
    return (np.ascontiguousarray(lhsT), np.ascontiguousarray(rhs),
            np.ascontiguousarray(lhsT2), np.ascontiguousarray(rhs2),
            head)


def _prep(pc):
    """Sort, quantize, window, and encode all batches."""
    pc = np.asarray(pc, dtype=np.float32)
    perms, xqs, sqs, xs_list = [], [], [], []
    for b in range(B):
        perm = np.argsort(pc[b][:, 0], kind="stable")
        xs = pc[b][perm].astype(np.float64)
        xq = np.round(xs * (2**GBITS))
        assert np.abs(xq).max() <= 2047
        perms.append(perm)
        xqs.append(xq)
        sqs.append((xq * xq).sum(-1).astype(np.int64))
        xs_list.append(xs[:, 0])
    offs, widths = _windows_from_sorted(xs_list)
    return perms, xqs, sqs, offs, widths


def run_device(pc, trace: bool = False):
    """Returns (list of per-core t16 [N,K] f32 packed winners, results,
    per-batch perms, per-batch xq)."""
    perms, xqs, sqs, offs, widths = _prep(pc)
    in_maps = []
    for b in range(B):
        lhsT, rhs, lhsT2, rhs2, head = _encode(xqs[b], sqs[b])
        in_maps.append({"lhsT": lhsT, "rhs": rhs, "lhsT2": lhsT2,
                        "rhs2": rhs2, "head": head})
    nc = _get_program(offs, widths)
    res = run_bass_kernel_spmd(nc, in_maps, core_ids=list(range(B)), trace=trace)
    t16s = [res.results[b]["t16"] for b in range(B)]
    return t16s, res, perms, xqs


def kernel(pc: np.ndarray, flow: np.ndarray) -> np.ndarray:
    pc = np.asarray(pc, dtype=np.float32)
    flow = np.asarray(flow, dtype=np.float32)
    t16s, _, perms, xqs = run_device(pc)
    total = 0.0
    rid = np.arange(N, dtype=np.int64)
    for b in range(B):
        cand = t16s[b].astype(np.float64)  # [N, NCAND]
        w64 = -np.partition(-cand, K - 1, axis=1)[:, :K]  # top-16 of 24
        w64 = np.sort(w64, axis=1)[:, ::-1]
        wg = np.floor(w64 * 2.0**16) * 2.0**-16
        jrec = np.round((w64 - wg) * 2.0**28).astype(np.int64)
        sel = w64 > 0
        res = np.where(sel, np.clip(jrec, 0, N - 1), rid[:, None])
        res[:, 0] = rid
        # grid-coincident pairs tie at w=R^2 where idx bits no longer fit in
        # f32; restore both partners exactly.
        xq = xqs[b].astype(np.int64)
        key = ((xq[:, 0] + 4096) << 26) + ((xq[:, 1] + 4096) << 13) + (xq[:, 2] + 4096)
        order = np.argsort(key, kind="stable")
        ks = key[order]
        for t in np.nonzero(ks[1:] == ks[:-1])[0]:
            i, j = order[t], order[t + 1]
            res[i, 1] = j
            res[j, 1] = i
        fs = flow[b][perms[b]].astype(np.float64)
        nn = fs[res]
        total += np.abs(fs[:, None, :] - nn).sum()
    return np.float32(total / (B * N * K))
